# revision 16
# baseline (speedup 1.0000x reference)
"""Trainium2 Bass kernel for a GNN message-passing layer (gather-free).

Reference computation (per node n, neighbors k=0..31):
  sa = src_atom_emb[atomic]            [N,128]
  ta = tgt_atom_emb[atomic]            [N,128]
  sd = silu(nde @ src_dir_W + b)       [N,64]
  td = silu(nde @ tgt_dir_W + b)       [N,64]
  edist = silu(ede @ dist_W + b)       [N,K,128]
  feat  = [edist | sd[nbr] | sa[nbr] | td | ta]   [N,K,512]
  out   = sum_k(mask*feat) / (sum_k mask + 1e-5)  [N,512]

Strategy (8 cores, nodes sharded 1250/core, SPMD, no collectives, no
on-device gather):
  - Host compacts each core's valid edges into a degree-sorted stream
    (node runs padded to even length, canonical run lengths shared by
    all 8 cores so one program serves all).  Per edge the host ships
    the fp16 ede row AND the fp16 nde row of the *source* node (plus a
    validity/bias lane), so the neighbor gather becomes pure host-side
    data staging like the baseline's premasking.
  - dist branch: fp16 PE matmul of dist_W against the edge stream,
    ACT silu into a big fp32 SBUF buffer, per-degree-class DVE reduce.
  - sd branch: two edges are packed per moving column (block-diagonal
    duplicated weights), silu into an fp16 buffer, class reduce, then
    one fold add of the two partition halves.
  - sa[nbr] sum: host builds a per-node histogram over the 100 atom
    types of its valid neighbors; on-chip one fp16 matmul against the
    src embedding per 512 nodes reproduces the masked gather-sum
    exactly (counts are exact in fp16).
  - ta / td: one-hot and direction matmuls per 512 nodes (td in fp32,
    its values are too large for fp16 rounding at the 2e-2 gate).
  - Output stays [512 dims, nodes]; host transposes, scales by
    1/(cnt+1e-5) (cnt/(cnt+1e-5) for the receiver block) and undoes
    the degree sort.  All learned-layer FLOPs stay on device.
"""

import numpy as np

import sys

sys.path.insert(0, "/opt/trn_rl_repo")

import concourse.bacc as bacc  # noqa: E402
import concourse.bass as bass  # noqa: E402,F401
import concourse.mybir as mybir  # noqa: E402
import concourse.tile as tile  # noqa: E402
from concourse.bass_utils import run_bass_kernel_spmd  # noqa: E402

# Problem shape (hardcoded; harness always uses these).
N_CORES = 8
N = 10000
K = 32
NLOC = N // N_CORES          # 1250 nodes per core
NPAD = 1280                  # padded node count (multiple of 512 slices ok)
D_DIR_IN = 10
NUM_ELEM = 100
FP32 = mybir.dt.float32
FP16 = mybir.dt.float16

_CACHED = {}


def _pieces_by_tile(classes, NT, tile=2048):
    """Split each degree class into node ranges that complete within each
    silu tile, so reduces can interleave with the edge stream.  Returns
    {tile_j: [(d, node_start, n_nodes, edge_off), ...]} plus per-tile
    completed-node watermark."""
    by_tile = {j: [] for j in range(NT)}
    done_nodes = [0] * NT
    for (d, s, n, off) in classes:
        prev = s
        j = max(0, (off + d - 1) // tile)
        while prev < s + n:
            # last node whose run ends within tiles 0..j
            u = s + min(n, ((j + 1) * tile - off) // d)
            if u - prev >= 4 or u == s + n:
                if u > prev:
                    by_tile[min(j, NT - 1)].append(
                        (d, prev, u - prev, off + (prev - s) * d)
                    )
                    prev = u
            j += 1
            if j >= NT and prev < s + n:
                by_tile[NT - 1].append(
                    (d, prev, s + n - prev, off + (prev - s) * d)
                )
                prev = s + n
    last = 0
    for j in range(NT):
        for (d, s2, n2, off2) in by_tile[j]:
            last = max(last, s2 + n2)
        done_nodes[j] = last
    return by_tile, done_nodes


def _build_program(ECp, classes):
    """classes: tuple of (d, node_start, n_nodes, edge_off); covers all NPAD
    nodes with even run length d >= 2 and sum(d*n) == EC <= ECp."""
    NT = ECp // 2048             # ede stream tiles
    EC2p = ECp // 2              # parity-packed sd columns

    nc = bacc.Bacc(
        "TRN2",
        target_bir_lowering=False,
        debug=False,
        enable_asserts=False,
        num_devices=N_CORES,
    )

    edeC = nc.dram_tensor("edeC", [128, ECp], FP16, kind="ExternalInput")
    nde2 = nc.dram_tensor("nde2", [24, EC2p], FP16, kind="ExternalInput")
    w_dist = nc.dram_tensor("w_dist", [128, 128], FP16, kind="ExternalInput")
    w_sd2 = nc.dram_tensor("w_sd2", [24, 128], FP16, kind="ExternalInput")
    w_td2 = nc.dram_tensor("w_td2", [12, 64], FP32, kind="ExternalInput")
    ndeTl = nc.dram_tensor("ndeTl", [12, NPAD], FP32, kind="ExternalInput")
    histT = nc.dram_tensor("histT", [128, NPAD], FP16, kind="ExternalInput")
    ohT = nc.dram_tensor("ohT", [128, NPAD], FP16, kind="ExternalInput")
    emb_s = nc.dram_tensor("emb_s", [128, 128], FP16, kind="ExternalInput")
    emb_t = nc.dram_tensor("emb_t", [128, 128], FP16, kind="ExternalInput")
    outT = nc.dram_tensor("outT", [512, NLOC], FP32, kind="ExternalOutput")
    sdB = nc.dram_tensor("sdB", [64, NLOC], FP32, kind="ExternalOutput")

    Silu = mybir.ActivationFunctionType.Silu
    Add = mybir.AluOpType.add
    X = mybir.AxisListType.X

    by_tile, done_nodes = _pieces_by_tile(classes, NT)

    with tile.TileContext(nc) as tc:
        from contextlib import ExitStack

        with ExitStack() as ctx:
            const = ctx.enter_context(tc.tile_pool(name="const", bufs=1))
            acc = ctx.enter_context(tc.tile_pool(name="acc", bufs=1))
            ede_pool = ctx.enter_context(tc.tile_pool(name="ede_pool", bufs=6))
            pd = ctx.enter_context(tc.tile_pool(name="pd", bufs=2, space="PSUM"))
            psd = ctx.enter_context(tc.tile_pool(name="psd", bufs=1, space="PSUM"))
            pasm = ctx.enter_context(tc.tile_pool(name="pasm", bufs=1, space="PSUM"))

            # --- constants: stream-critical ones on SP, the rest on Pool so
            # the edge stream's loads lead the SP queue ---
            w_dist_s = const.tile([128, 128], FP16)
            nc.sync.dma_start(w_dist_s[:], w_dist[:, :])
            w_sd2_s = const.tile([24, 128], FP16)
            nc.sync.dma_start(w_sd2_s[:], w_sd2[:, :])
            histT_s = const.tile([128, NPAD], FP16)
            nc.gpsimd.dma_start(histT_s[:], histT[:, :])
            ohT_s = const.tile([128, NPAD], FP16)
            nc.gpsimd.dma_start(ohT_s[:], ohT[:, :])
            emb_s_s = const.tile([128, 128], FP16)
            nc.gpsimd.dma_start(emb_s_s[:], emb_s[:, :])
            emb_t_s = const.tile([128, 128], FP16)
            nc.gpsimd.dma_start(emb_t_s[:], emb_t[:, :])
            nde2_s = const.tile([24, EC2p], FP16)
            nc.gpsimd.dma_start(nde2_s[:], nde2[:, :])
            ndeTl_s = const.tile([12, NPAD], FP32)
            nc.gpsimd.dma_start(ndeTl_s[:], ndeTl[:, :])
            w_td2_s = const.tile([12, 64], FP32)
            nc.gpsimd.dma_start(w_td2_s[:], w_td2[:, :])

            # --- sa/ta matmuls first: they fill the PE while the first ede
            # tiles are still in flight; copies go on the early-idle DVE ---
            sa_acc = acc.tile([128, NPAD], FP32)
            ta_acc = acc.tile([128, NPAD], FP32)
            td_acc = acc.tile([64, NPAD], FP32)
            for t in range(3):
                c0 = t * 512
                cols = min(512, NPAD - c0)
                ps_sa = pasm.tile([128, 512], FP32, tag="o")
                nc.tensor.matmul(
                    ps_sa[:, :cols], emb_s_s[:], histT_s[:, c0 : c0 + cols],
                    start=True, stop=True,
                )
                nc.vector.tensor_copy(sa_acc[:, c0 : c0 + cols], ps_sa[:, :cols])
                ps_ta = pasm.tile([128, 512], FP32, tag="o")
                nc.tensor.matmul(
                    ps_ta[:, :cols], emb_t_s[:], ohT_s[:, c0 : c0 + cols],
                    start=True, stop=True,
                )
                nc.vector.tensor_copy(ta_acc[:, c0 : c0 + cols], ps_ta[:, :cols])
            nc.gpsimd.dma_start(outT[192:320, :], sa_acc[:, :NLOC])
            nc.gpsimd.dma_start(outT[384:512, :], ta_acc[:, :NLOC])

            # --- edge streams + interleaved reduces + chunked output ---
            dist_silu = acc.tile([128, ECp], FP32)
            sd_silu = acc.tile([128, EC2p], FP16)
            dist_acc = acc.tile([128, NPAD], FP32)
            sd_acc = acc.tile([128, NPAD], FP32)
            out_done = 0
            for j in range(NT):
                t_ede = ede_pool.tile([128, 2048], FP16)
                nc.sync.dma_start(t_ede[:], edeC[:, j * 2048 : (j + 1) * 2048])
                psdt = psd.tile([128, 1024], FP32)
                nc.tensor.matmul(
                    psdt[:, :512], w_sd2_s[:], nde2_s[:, j * 1024 : j * 1024 + 512],
                    start=True, stop=True,
                )
                nc.tensor.matmul(
                    psdt[:, 512:], w_sd2_s[:],
                    nde2_s[:, j * 1024 + 512 : j * 1024 + 1024],
                    start=True, stop=True,
                )
                nc.scalar.activation(
                    sd_silu[:, j * 1024 : (j + 1) * 1024], psdt[:], Silu
                )
                for h in range(2):
                    pdt = pd.tile([128, 1024], FP32)
                    base = h * 1024
                    nc.tensor.matmul(
                        pdt[:, :512], w_dist_s[:], t_ede[:, base : base + 512],
                        start=True, stop=True,
                    )
                    nc.tensor.matmul(
                        pdt[:, 512:], w_dist_s[:], t_ede[:, base + 512 : base + 1024],
                        start=True, stop=True,
                    )
                    nc.scalar.activation(
                        dist_silu[:, j * 2048 + base : j * 2048 + base + 1024],
                        pdt[:], Silu,
                    )
                for (d, s, n, off) in by_tile[j]:
                    nc.vector.tensor_reduce(
                        sd_acc[:, s : s + n],
                        sd_silu[:, off // 2 : off // 2 + n * (d // 2)].rearrange(
                            "p (n k) -> p n k", k=d // 2
                        ),
                        X, Add,
                    )
                    nc.vector.tensor_reduce(
                        dist_acc[:, s : s + n],
                        dist_silu[:, off : off + n * d].rearrange(
                            "p (n k) -> p n k", k=d
                        ),
                        X, Add,
                    )
                # flush completed node columns to DRAM in chunks
                w = min(done_nodes[j], NLOC)
                if j < NT - 1 and w - out_done >= 256:
                    nc.gpsimd.dma_start(
                        outT[0:128, out_done:w], dist_acc[:, out_done:w]
                    )
                    nc.gpsimd.dma_start(
                        outT[128:192, out_done:w], sd_acc[0:64, out_done:w]
                    )
                    nc.gpsimd.dma_start(
                        sdB[:, out_done:w], sd_acc[64:128, out_done:w]
                    )
                    out_done = w
            # final flush on the (now idle) ACT hardware-DGE queue
            nc.scalar.dma_start(outT[0:128, out_done:NLOC], dist_acc[:, out_done:NLOC])
            nc.scalar.dma_start(
                outT[128:192, out_done:NLOC], sd_acc[0:64, out_done:NLOC]
            )
            nc.scalar.dma_start(sdB[:, out_done:NLOC], sd_acc[64:128, out_done:NLOC])

            # --- td branch (fp32 matmuls) rides the DVE-reduce tail ---
            for t in range(3):
                c0 = t * 512
                cols = min(512, NPAD - c0)
                ps_td = pasm.tile([64, 512], FP32, tag="td")
                nc.tensor.matmul(
                    ps_td[:, :cols], w_td2_s[:], ndeTl_s[:, c0 : c0 + cols],
                    start=True, stop=True,
                )
                nc.scalar.activation(td_acc[:, c0 : c0 + cols], ps_td[:, :cols], Silu)
            nc.gpsimd.dma_start(outT[320:384, :], td_acc[:, :NLOC])

    nc.compile()
    return nc


def _prep_core(c, atomic, nde, ede, nbr, mask, DP, offs, ECp):
    """Build one core's device arrays given the canonical run lengths DP."""
    f16 = np.float16
    lo, hi = c * NLOC, (c + 1) * NLOC
    a_loc = atomic[lo:hi]
    nde_loc = nde[lo:hi]
    ede_loc = ede[lo:hi]
    nbr_loc = nbr[lo:hi]
    mask_loc = mask[lo:hi]

    deg = mask_loc.sum(1).astype(np.int64)
    dp0 = np.maximum(2, ((deg + 1) // 2) * 2)
    order = np.argsort(-dp0, kind="stable")          # sorted -> old local idx

    ml_sorted = mask_loc[order]
    deg_sorted = deg[order]
    nz_i, nz_k = np.nonzero(ml_sorted)               # grouped by sorted node
    E = nz_i.shape[0]
    grp_start = np.zeros(NLOC, np.int64)
    grp_start[1:] = np.cumsum(deg_sorted)[:-1]
    pos = offs[nz_i] + (np.arange(E) - grp_start[nz_i])

    src = nbr_loc[order][nz_i, nz_k]                 # global source node ids

    edeR = np.zeros((ECp, 128), f16)
    edeR[pos] = ede_loc[order][nz_i, nz_k].astype(f16)
    edeC = np.ascontiguousarray(edeR.T)

    ndeE = np.zeros((ECp, 12), np.float32)
    ndeE[pos, :D_DIR_IN] = nde[src]
    ndeE[pos, D_DIR_IN] = 1.0
    nde2 = np.ascontiguousarray(
        ndeE.reshape(ECp // 2, 24).T.astype(f16)
    )

    histT = np.zeros((128, NPAD), np.float32)
    np.add.at(histT, (atomic[src], nz_i), 1.0)

    ohT = np.zeros((128, NPAD), f16)
    ohT[a_loc[order], np.arange(NLOC)] = 1.0

    ndeTl = np.zeros((12, NPAD), np.float32)
    ndeTl[:D_DIR_IN, :NLOC] = nde_loc[order].T
    ndeTl[D_DIR_IN, :NLOC] = 1.0

    return {
        "edeC": edeC,
        "nde2": nde2,
        "histT": histT.astype(f16),
        "ohT": ohT,
        "ndeTl": ndeTl,
    }, order, deg_sorted


def _prepare_all(inputs):
    f32 = np.float32
    atomic = np.asarray(inputs["atomic_numbers"]).astype(np.int64)
    nde = np.asarray(inputs["node_direction_expansion"]).astype(f32)
    ede = np.asarray(inputs["edge_distance_expansion"]).astype(f32)
    nbr = np.asarray(inputs["neighbor_list"]).astype(np.int64)
    mask = np.asarray(inputs["neighbor_mask"]).astype(bool)
    emb_s = np.asarray(inputs["src_atom_emb"]).astype(f32)
    emb_t = np.asarray(inputs["tgt_atom_emb"]).astype(f32)
    w_sd = np.asarray(inputs["src_dir_W"]).astype(f32)
    b_sd = np.asarray(inputs["src_dir_b"]).astype(f32)
    w_td = np.asarray(inputs["tgt_dir_W"]).astype(f32)
    b_td = np.asarray(inputs["tgt_dir_b"]).astype(f32)
    w_di = np.asarray(inputs["dist_W"]).astype(f32)
    b_di = np.asarray(inputs["dist_b"]).astype(f32)
    assert np.all(b_di == 0.0), "nonzero dist_b not supported"

    # canonical per-position run lengths across cores (shared program)
    deg_all = mask.reshape(N_CORES, NLOC, K).sum(2).astype(np.int64)
    dp0 = np.maximum(2, ((deg_all + 1) // 2) * 2)
    dp_sorted = -np.sort(-dp0, axis=1)
    DP = np.concatenate(
        [dp_sorted.max(0), np.full(NPAD - NLOC, 2, np.int64)]
    )
    offs = np.zeros(NPAD + 1, np.int64)
    offs[1:] = np.cumsum(DP)
    EC = int(offs[NPAD])
    ECp = ((EC + 2047) // 2048) * 2048

    classes = []
    i = 0
    while i < NPAD:
        j = i
        while j < NPAD and DP[j] == DP[i]:
            j += 1
        classes.append((int(DP[i]), i, j - i, int(offs[i])))
        i = j
    classes = tuple(classes)

    f16 = np.float16
    W12 = np.zeros((12, 64), f32)
    W12[:D_DIR_IN] = w_sd
    W12[D_DIR_IN] = b_sd
    w_sd2 = np.zeros((24, 128), f16)
    w_sd2[:12, :64] = W12.astype(f16)
    w_sd2[12:, 64:] = W12.astype(f16)
    W12t = np.zeros((12, 64), f32)
    W12t[:D_DIR_IN] = w_td
    W12t[D_DIR_IN] = b_td
    emb_s_pad = np.zeros((128, 128), f16)
    emb_s_pad[:NUM_ELEM] = emb_s.astype(f16)
    emb_t_pad = np.zeros((128, 128), f16)
    emb_t_pad[:NUM_ELEM] = emb_t.astype(f16)

    shared = {
        "w_dist": np.ascontiguousarray(w_di.astype(f16)),
        "w_sd2": w_sd2,
        "w_td2": np.ascontiguousarray(W12t),
        "emb_s": emb_s_pad,
        "emb_t": emb_t_pad,
    }

    in_maps = []
    posts = []
    for c in range(N_CORES):
        m, order, deg_sorted = _prep_core(
            c, atomic, nde, ede, nbr, mask, DP, offs, ECp
        )
        m.update(shared)
        in_maps.append(m)
        posts.append((order, deg_sorted))
    return in_maps, posts, ECp, classes


def _run(inputs, trace=False, **spmd_kwargs):
    in_maps, posts, ECp, classes = _prepare_all(inputs)
    key = (ECp, classes)
    if key not in _CACHED:
        _CACHED[key] = _build_program(ECp, classes)
    nc = _CACHED[key]

    res = run_bass_kernel_spmd(
        nc, in_maps, list(range(N_CORES)), trace=trace, **spmd_kwargs
    )
    outs = []
    for c in range(N_CORES):
        raw = np.asarray(res.results[c]["outT"], np.float32)   # [512, NLOC]
        sdb = np.asarray(res.results[c]["sdB"], np.float32)    # [64, NLOC]
        order, deg_sorted = posts[c]
        o = np.ascontiguousarray(raw.T)                         # sorted nodes
        o[:, 128:192] += sdb.T
        inv = 1.0 / (deg_sorted.astype(np.float32) + 1e-5)
        cim = deg_sorted.astype(np.float32) * inv
        o[:, :320] *= inv[:, None]
        o[:, 320:] *= cim[:, None]
        final = np.empty((NLOC, 512), np.float32)
        final[order] = o
        outs.append(final)
    out = np.concatenate(outs, axis=0)
    return out, res


def kernel(**inputs):
    out, _ = _run(inputs, trace=False)
    return out


# revision 17
# speedup vs baseline: 1.0684x; 1.0684x over previous
"""Trainium2 Bass kernel for a GNN message-passing layer (gather-free).

Reference computation (per node n, neighbors k=0..31):
  sa = src_atom_emb[atomic]            [N,128]
  ta = tgt_atom_emb[atomic]            [N,128]
  sd = silu(nde @ src_dir_W + b)       [N,64]
  td = silu(nde @ tgt_dir_W + b)       [N,64]
  edist = silu(ede @ dist_W + b)       [N,K,128]
  feat  = [edist | sd[nbr] | sa[nbr] | td | ta]   [N,K,512]
  out   = sum_k(mask*feat) / (sum_k mask + 1e-5)  [N,512]

Strategy (8 cores, nodes sharded 1250/core, SPMD, no collectives, no
on-device gather):
  - Host compacts each core's valid edges into a degree-sorted stream
    (node runs padded to even length, canonical run lengths shared by
    all 8 cores so one program serves all).  Per edge the host ships
    the fp16 ede row AND the fp16 nde row of the *source* node (plus a
    validity/bias lane), so the neighbor gather becomes pure host-side
    data staging like the baseline's premasking.
  - dist branch: fp16 PE matmul of dist_W against the edge stream,
    ACT silu into a big fp32 SBUF buffer, per-degree-class DVE reduce.
  - sd branch: two edges are packed per moving column (block-diagonal
    duplicated weights), silu into an fp16 buffer, class reduce, then
    one fold add of the two partition halves.
  - sa[nbr] sum: host builds a per-node histogram over the 100 atom
    types of its valid neighbors; on-chip one fp16 matmul against the
    src embedding per 512 nodes reproduces the masked gather-sum
    exactly (counts are exact in fp16).
  - ta / td: one-hot and direction matmuls per 512 nodes (td in fp32,
    its values are too large for fp16 rounding at the 2e-2 gate).
  - Output stays [512 dims, nodes]; host transposes, scales by
    1/(cnt+1e-5) (cnt/(cnt+1e-5) for the receiver block) and undoes
    the degree sort.  All learned-layer FLOPs stay on device.
"""

import numpy as np

import sys

sys.path.insert(0, "/opt/trn_rl_repo")

import concourse.bacc as bacc  # noqa: E402
import concourse.bass as bass  # noqa: E402,F401
import concourse.mybir as mybir  # noqa: E402
import concourse.tile as tile  # noqa: E402
from concourse.bass_utils import run_bass_kernel_spmd  # noqa: E402

# Problem shape (hardcoded; harness always uses these).
N_CORES = 8
N = 10000
K = 32
NLOC = N // N_CORES          # 1250 nodes per core
NPAD = 1280                  # padded node count (multiple of 512 slices ok)
D_DIR_IN = 10
NUM_ELEM = 100
FP32 = mybir.dt.float32
FP16 = mybir.dt.float16

_CACHED = {}


def _pieces_by_tile(classes, NT, tile=2048):
    """Split each degree class into node ranges that complete within each
    silu tile, so reduces can interleave with the edge stream.  Returns
    {tile_j: [(d, node_start, n_nodes, edge_off), ...]} plus per-tile
    completed-node watermark."""
    by_tile = {j: [] for j in range(NT)}
    done_nodes = [0] * NT
    for (d, s, n, off) in classes:
        prev = s
        j = max(0, (off + d - 1) // tile)
        while prev < s + n:
            # last node whose run ends within tiles 0..j
            u = s + min(n, ((j + 1) * tile - off) // d)
            if u - prev >= 4 or u == s + n:
                if u > prev:
                    by_tile[min(j, NT - 1)].append(
                        (d, prev, u - prev, off + (prev - s) * d)
                    )
                    prev = u
            j += 1
            if j >= NT and prev < s + n:
                by_tile[NT - 1].append(
                    (d, prev, s + n - prev, off + (prev - s) * d)
                )
                prev = s + n
    last = 0
    for j in range(NT):
        for (d, s2, n2, off2) in by_tile[j]:
            last = max(last, s2 + n2)
        done_nodes[j] = last
    return by_tile, done_nodes


def _build_program(ECp, classes):
    """classes: tuple of (d, node_start, n_nodes, edge_off); covers all NPAD
    nodes with even run length d >= 2 and sum(d*n) == EC <= ECp."""
    NT = ECp // 2048             # ede stream tiles
    EC2p = ECp // 2              # parity-packed sd columns

    nc = bacc.Bacc(
        "TRN2",
        target_bir_lowering=False,
        debug=False,
        enable_asserts=False,
        num_devices=N_CORES,
    )

    edeC = nc.dram_tensor("edeC", [128, ECp], FP16, kind="ExternalInput")
    nde2 = nc.dram_tensor("nde2", [24, EC2p], FP16, kind="ExternalInput")
    w_dist = nc.dram_tensor("w_dist", [128, 128], FP16, kind="ExternalInput")
    w_sd2 = nc.dram_tensor("w_sd2", [24, 128], FP16, kind="ExternalInput")
    w_td2 = nc.dram_tensor("w_td2", [12, 64], FP32, kind="ExternalInput")
    ndeTl = nc.dram_tensor("ndeTl", [12, NPAD], FP32, kind="ExternalInput")
    histT = nc.dram_tensor("histT", [128, NPAD], FP16, kind="ExternalInput")
    ohT = nc.dram_tensor("ohT", [128, NPAD], FP16, kind="ExternalInput")
    emb_s = nc.dram_tensor("emb_s", [128, 128], FP16, kind="ExternalInput")
    emb_t = nc.dram_tensor("emb_t", [128, 128], FP16, kind="ExternalInput")
    outT = nc.dram_tensor("outT", [512, NLOC], FP32, kind="ExternalOutput")
    sdB = nc.dram_tensor("sdB", [64, NLOC], FP32, kind="ExternalOutput")

    Silu = mybir.ActivationFunctionType.Silu
    Add = mybir.AluOpType.add
    X = mybir.AxisListType.X

    by_tile, done_nodes = _pieces_by_tile(classes, NT)

    with tile.TileContext(nc) as tc:
        from contextlib import ExitStack

        with ExitStack() as ctx:
            const = ctx.enter_context(tc.tile_pool(name="const", bufs=1))
            acc = ctx.enter_context(tc.tile_pool(name="acc", bufs=1))
            ede_pool = ctx.enter_context(tc.tile_pool(name="ede_pool", bufs=6))
            pd = ctx.enter_context(tc.tile_pool(name="pd", bufs=2, space="PSUM"))
            psd = ctx.enter_context(tc.tile_pool(name="psd", bufs=1, space="PSUM"))
            pasm = ctx.enter_context(tc.tile_pool(name="pasm", bufs=1, space="PSUM"))

            # --- constants: stream-critical ones on SP, the rest on Pool so
            # the edge stream's loads lead the SP queue ---
            w_dist_s = const.tile([128, 128], FP16)
            nc.sync.dma_start(w_dist_s[:], w_dist[:, :])
            w_sd2_s = const.tile([24, 128], FP16)
            nc.sync.dma_start(w_sd2_s[:], w_sd2[:, :])
            histT_s = const.tile([128, NPAD], FP16)
            nc.gpsimd.dma_start(histT_s[:], histT[:, :])
            ohT_s = const.tile([128, NPAD], FP16)
            nc.gpsimd.dma_start(ohT_s[:], ohT[:, :])
            emb_s_s = const.tile([128, 128], FP16)
            nc.gpsimd.dma_start(emb_s_s[:], emb_s[:, :])
            emb_t_s = const.tile([128, 128], FP16)
            nc.gpsimd.dma_start(emb_t_s[:], emb_t[:, :])
            nde2_s = const.tile([24, EC2p], FP16)
            nc.gpsimd.dma_start(nde2_s[:], nde2[:, :])
            ndeTl_s = const.tile([12, NPAD], FP32)
            nc.gpsimd.dma_start(ndeTl_s[:], ndeTl[:, :])
            w_td2_s = const.tile([12, 64], FP32)
            nc.gpsimd.dma_start(w_td2_s[:], w_td2[:, :])

            # --- sa/ta matmuls first: they fill the PE while the first ede
            # tiles are still in flight; copies go on the early-idle DVE ---
            sa_acc = acc.tile([128, NPAD], FP32)
            ta_acc = acc.tile([128, NPAD], FP32)
            td_acc = acc.tile([64, NPAD], FP32)
            for t in range(3):
                c0 = t * 512
                cols = min(512, NPAD - c0)
                ps_sa = pasm.tile([128, 512], FP32, tag="o")
                nc.tensor.matmul(
                    ps_sa[:, :cols], emb_s_s[:], histT_s[:, c0 : c0 + cols],
                    start=True, stop=True,
                )
                nc.vector.tensor_copy(sa_acc[:, c0 : c0 + cols], ps_sa[:, :cols])
                ps_ta = pasm.tile([128, 512], FP32, tag="o")
                nc.tensor.matmul(
                    ps_ta[:, :cols], emb_t_s[:], ohT_s[:, c0 : c0 + cols],
                    start=True, stop=True,
                )
                nc.vector.tensor_copy(ta_acc[:, c0 : c0 + cols], ps_ta[:, :cols])
            nc.gpsimd.dma_start(outT[192:320, :], sa_acc[:, :NLOC])
            nc.gpsimd.dma_start(outT[384:512, :], ta_acc[:, :NLOC])

            # --- edge streams + interleaved reduces + chunked output ---
            dist_silu = acc.tile([128, ECp], FP32)
            sd_silu = acc.tile([128, EC2p], FP16)
            dist_acc = acc.tile([128, NPAD], FP32)
            sd_acc = acc.tile([128, NPAD], FP32)
            out_done = 0
            for j in range(NT):
                t_ede = ede_pool.tile([128, 2048], FP16)
                nc.sync.dma_start(t_ede[:], edeC[:, j * 2048 : (j + 1) * 2048])
                for h in range(2):
                    pdt = pd.tile([128, 1024], FP32)
                    base = h * 1024
                    nc.tensor.matmul(
                        pdt[:, :512], w_dist_s[:], t_ede[:, base : base + 512],
                        start=True, stop=True,
                    )
                    nc.tensor.matmul(
                        pdt[:, 512:], w_dist_s[:], t_ede[:, base + 512 : base + 1024],
                        start=True, stop=True,
                    )
                    nc.scalar.activation(
                        dist_silu[:, j * 2048 + base : j * 2048 + base + 1024],
                        pdt[:], Silu,
                    )
                psdt = psd.tile([128, 1024], FP32)
                nc.tensor.matmul(
                    psdt[:, :512], w_sd2_s[:], nde2_s[:, j * 1024 : j * 1024 + 512],
                    start=True, stop=True,
                )
                nc.tensor.matmul(
                    psdt[:, 512:], w_sd2_s[:],
                    nde2_s[:, j * 1024 + 512 : j * 1024 + 1024],
                    start=True, stop=True,
                )
                nc.scalar.activation(
                    sd_silu[:, j * 1024 : (j + 1) * 1024], psdt[:], Silu
                )
                for (d, s, n, off) in by_tile[j]:
                    nc.vector.tensor_reduce(
                        sd_acc[:, s : s + n],
                        sd_silu[:, off // 2 : off // 2 + n * (d // 2)].rearrange(
                            "p (n k) -> p n k", k=d // 2
                        ),
                        X, Add,
                    )
                    nc.vector.tensor_reduce(
                        dist_acc[:, s : s + n],
                        dist_silu[:, off : off + n * d].rearrange(
                            "p (n k) -> p n k", k=d
                        ),
                        X, Add,
                    )
                # flush completed node columns to DRAM in chunks
                w = min(done_nodes[j], NLOC)
                if j < NT - 1 and w - out_done >= 256:
                    nc.gpsimd.dma_start(
                        outT[0:128, out_done:w], dist_acc[:, out_done:w]
                    )
                    nc.gpsimd.dma_start(
                        outT[128:192, out_done:w], sd_acc[0:64, out_done:w]
                    )
                    nc.gpsimd.dma_start(
                        sdB[:, out_done:w], sd_acc[64:128, out_done:w]
                    )
                    out_done = w

            # --- td branch (fp32 matmuls) rides the DVE-reduce tail ---
            for t in range(3):
                c0 = t * 512
                cols = min(512, NPAD - c0)
                ps_td = pasm.tile([64, 512], FP32, tag="td")
                nc.tensor.matmul(
                    ps_td[:, :cols], w_td2_s[:], ndeTl_s[:, c0 : c0 + cols],
                    start=True, stop=True,
                )
                nc.scalar.activation(td_acc[:, c0 : c0 + cols], ps_td[:, :cols], Silu)
            nc.gpsimd.dma_start(outT[320:384, :], td_acc[:, :NLOC])
            # final flush on the (now idle) ACT hardware-DGE queue
            nc.scalar.dma_start(outT[0:128, out_done:NLOC], dist_acc[:, out_done:NLOC])
            nc.scalar.dma_start(
                outT[128:192, out_done:NLOC], sd_acc[0:64, out_done:NLOC]
            )
            nc.scalar.dma_start(sdB[:, out_done:NLOC], sd_acc[64:128, out_done:NLOC])

    nc.compile()
    return nc


def _prep_core(c, atomic, nde, ede, nbr, mask, DP, offs, ECp):
    """Build one core's device arrays given the canonical run lengths DP."""
    f16 = np.float16
    lo, hi = c * NLOC, (c + 1) * NLOC
    a_loc = atomic[lo:hi]
    nde_loc = nde[lo:hi]
    ede_loc = ede[lo:hi]
    nbr_loc = nbr[lo:hi]
    mask_loc = mask[lo:hi]

    deg = mask_loc.sum(1).astype(np.int64)
    dp0 = np.maximum(2, ((deg + 1) // 2) * 2)
    order = np.argsort(-dp0, kind="stable")          # sorted -> old local idx

    ml_sorted = mask_loc[order]
    deg_sorted = deg[order]
    nz_i, nz_k = np.nonzero(ml_sorted)               # grouped by sorted node
    E = nz_i.shape[0]
    grp_start = np.zeros(NLOC, np.int64)
    grp_start[1:] = np.cumsum(deg_sorted)[:-1]
    pos = offs[nz_i] + (np.arange(E) - grp_start[nz_i])

    src = nbr_loc[order][nz_i, nz_k]                 # global source node ids

    edeR = np.zeros((ECp, 128), f16)
    edeR[pos] = ede_loc[order][nz_i, nz_k].astype(f16)
    edeC = np.ascontiguousarray(edeR.T)

    ndeE = np.zeros((ECp, 12), np.float32)
    ndeE[pos, :D_DIR_IN] = nde[src]
    ndeE[pos, D_DIR_IN] = 1.0
    nde2 = np.ascontiguousarray(
        ndeE.reshape(ECp // 2, 24).T.astype(f16)
    )

    histT = np.zeros((128, NPAD), np.float32)
    np.add.at(histT, (atomic[src], nz_i), 1.0)

    ohT = np.zeros((128, NPAD), f16)
    ohT[a_loc[order], np.arange(NLOC)] = 1.0

    ndeTl = np.zeros((12, NPAD), np.float32)
    ndeTl[:D_DIR_IN, :NLOC] = nde_loc[order].T
    ndeTl[D_DIR_IN, :NLOC] = 1.0

    return {
        "edeC": edeC,
        "nde2": nde2,
        "histT": histT.astype(f16),
        "ohT": ohT,
        "ndeTl": ndeTl,
    }, order, deg_sorted


def _prepare_all(inputs):
    f32 = np.float32
    atomic = np.asarray(inputs["atomic_numbers"]).astype(np.int64)
    nde = np.asarray(inputs["node_direction_expansion"]).astype(f32)
    ede = np.asarray(inputs["edge_distance_expansion"]).astype(f32)
    nbr = np.asarray(inputs["neighbor_list"]).astype(np.int64)
    mask = np.asarray(inputs["neighbor_mask"]).astype(bool)
    emb_s = np.asarray(inputs["src_atom_emb"]).astype(f32)
    emb_t = np.asarray(inputs["tgt_atom_emb"]).astype(f32)
    w_sd = np.asarray(inputs["src_dir_W"]).astype(f32)
    b_sd = np.asarray(inputs["src_dir_b"]).astype(f32)
    w_td = np.asarray(inputs["tgt_dir_W"]).astype(f32)
    b_td = np.asarray(inputs["tgt_dir_b"]).astype(f32)
    w_di = np.asarray(inputs["dist_W"]).astype(f32)
    b_di = np.asarray(inputs["dist_b"]).astype(f32)
    assert np.all(b_di == 0.0), "nonzero dist_b not supported"

    # canonical per-position run lengths across cores (shared program)
    deg_all = mask.reshape(N_CORES, NLOC, K).sum(2).astype(np.int64)
    dp0 = np.maximum(2, ((deg_all + 1) // 2) * 2)
    dp_sorted = -np.sort(-dp0, axis=1)
    DP = np.concatenate(
        [dp_sorted.max(0), np.full(NPAD - NLOC, 2, np.int64)]
    )
    offs = np.zeros(NPAD + 1, np.int64)
    offs[1:] = np.cumsum(DP)
    EC = int(offs[NPAD])
    ECp = ((EC + 2047) // 2048) * 2048

    classes = []
    i = 0
    while i < NPAD:
        j = i
        while j < NPAD and DP[j] == DP[i]:
            j += 1
        classes.append((int(DP[i]), i, j - i, int(offs[i])))
        i = j
    classes = tuple(classes)

    f16 = np.float16
    W12 = np.zeros((12, 64), f32)
    W12[:D_DIR_IN] = w_sd
    W12[D_DIR_IN] = b_sd
    w_sd2 = np.zeros((24, 128), f16)
    w_sd2[:12, :64] = W12.astype(f16)
    w_sd2[12:, 64:] = W12.astype(f16)
    W12t = np.zeros((12, 64), f32)
    W12t[:D_DIR_IN] = w_td
    W12t[D_DIR_IN] = b_td
    emb_s_pad = np.zeros((128, 128), f16)
    emb_s_pad[:NUM_ELEM] = emb_s.astype(f16)
    emb_t_pad = np.zeros((128, 128), f16)
    emb_t_pad[:NUM_ELEM] = emb_t.astype(f16)

    shared = {
        "w_dist": np.ascontiguousarray(w_di.astype(f16)),
        "w_sd2": w_sd2,
        "w_td2": np.ascontiguousarray(W12t),
        "emb_s": emb_s_pad,
        "emb_t": emb_t_pad,
    }

    in_maps = []
    posts = []
    for c in range(N_CORES):
        m, order, deg_sorted = _prep_core(
            c, atomic, nde, ede, nbr, mask, DP, offs, ECp
        )
        m.update(shared)
        in_maps.append(m)
        posts.append((order, deg_sorted))
    return in_maps, posts, ECp, classes


def _run(inputs, trace=False, **spmd_kwargs):
    in_maps, posts, ECp, classes = _prepare_all(inputs)
    key = (ECp, classes)
    if key not in _CACHED:
        _CACHED[key] = _build_program(ECp, classes)
    nc = _CACHED[key]

    res = run_bass_kernel_spmd(
        nc, in_maps, list(range(N_CORES)), trace=trace, **spmd_kwargs
    )
    outs = []
    for c in range(N_CORES):
        raw = np.asarray(res.results[c]["outT"], np.float32)   # [512, NLOC]
        sdb = np.asarray(res.results[c]["sdB"], np.float32)    # [64, NLOC]
        order, deg_sorted = posts[c]
        o = np.ascontiguousarray(raw.T)                         # sorted nodes
        o[:, 128:192] += sdb.T
        inv = 1.0 / (deg_sorted.astype(np.float32) + 1e-5)
        cim = deg_sorted.astype(np.float32) * inv
        o[:, :320] *= inv[:, None]
        o[:, 320:] *= cim[:, None]
        final = np.empty((NLOC, 512), np.float32)
        final[order] = o
        outs.append(final)
    out = np.concatenate(outs, axis=0)
    return out, res


def kernel(**inputs):
    out, _ = _run(inputs, trace=False)
    return out
